# revision 10
# baseline (speedup 1.0000x reference)
"""CoSTCo model kernel for 8x Trainium2 NeuronCores.

Math: out[b] = relu(wfc2 @ relu(wfc1 @ h2[b] + bfc1) + bfc2), where
  h2[b] = relu(Q02[i0[b]*64 + i2[b]] + Q1[i1[b]])
  Q_m   = relu(emb_m @ w1.T + b1) @ w2[:, :, m].T        (weight folding)
  Q02[i*64+j] = Q0[i] + Q2[j] + b2                       (pair fusion)

conv1 (over rank) and conv2 (over modes) act linearly on each gathered
embedding row, so they fold into per-table lookup matrices Q_m computed
once on the host; modes 0 and 2 fuse into one 21696-row pair table, so
the device does 2 dma_gathers per batch element instead of 3.

v5 structure (vs v1 baseline):
 - h2 relu drains moved to DVE (tensor_scalar max0; bf16 PSUM reads run
   near 2x rate) and h3 bias+relu drains to Scalar ACT, balancing the
   two elementwise engines under the ~73us gather wall
 - fc2 + final relu are emitted one block late so the PE never stalls
   waiting on the Scalar h3 drain (keeps the PE p-state clock high)
 - gathers are HBM random-read limited at the 512 B row granule
   (~230 GB/s); int8 rows measured no faster (granule halves, HBM
   efficiency halves) and cost extra DVE time + error, so tables stay
   bf16. Sorting by index measurably hurt; order stays random.

Sharding: pure data parallel over the batch dim, 16384 elements per core.
"""

import sys
import types

sys.path.insert(0, "/opt/trn_rl_repo")

import ml_dtypes
import numpy as np

# ---------------------------------------------------------------- constants
B = 131072
N_CORES = 8
BPC = B // N_CORES          # 16384 batch elements per core
CHUNK = 1024                # idx per dma_gather instruction
NCHUNK = BPC // CHUNK
C = 256                     # channels
FIELD_DIMS = (339, 5825, 64)
F02 = FIELD_DIMS[0] * FIELD_DIMS[2]   # fused pair-table rows
F1 = FIELD_DIMS[1]
NSWQ = 4                    # SWDGE queues (ucode max)

SORT02 = False              # sorting measurably hurt gather throughput


def _install_ntff_hook():
    """antenv in this image lacks axon_hooks; inject it and register the
    ctypes NTFF profiling hook so trace=True works under axon."""
    import antenv

    if "antenv.axon_hooks" in sys.modules:
        return
    mod = types.ModuleType("antenv.axon_hooks")
    mod._hook = None
    mod.set_axon_ntff_profile_hook = lambda h: setattr(mod, "_hook", h)
    mod.get_axon_ntff_profile_hook = lambda: mod._hook
    sys.modules["antenv.axon_hooks"] = mod
    antenv.axon_hooks = mod
    try:
        from trn_agent_boot.trn_boot import _ntff_profile_via_ctypes

        mod._hook = _ntff_profile_via_ctypes("/opt/axon/libaxon_pjrt.so")
    except Exception:
        pass


_NC_CACHE = {}


def _build(bpc=BPC, chunk=CHUNK):
    """Build + compile the per-core Bass program. Identical on all cores;
    per-core data arrives via in_maps."""
    import concourse.bass as bass
    import concourse.tile as tile
    from concourse import bacc, mybir

    key = (bpc, chunk)
    if key in _NC_CACHE:
        return _NC_CACHE[key]

    f32 = mybir.dt.float32
    bf16 = mybir.dt.bfloat16
    i16 = mybir.dt.int16
    i8 = mybir.dt.int8
    Alu = mybir.AluOpType
    Act = mybir.ActivationFunctionType
    nchunk = bpc // chunk
    nblk = chunk // 512
    ngrp = chunk // 128

    nc = bacc.Bacc("TRN2", target_bir_lowering=False, debug=False,
                   num_devices=N_CORES, num_swdge_queues=NSWQ,
                   dynamic_dma_scratch_size=65536)

    q02_dram = nc.dram_tensor("q02", [F02, C], bf16, kind="ExternalInput")
    q1_dram = nc.dram_tensor("q1", [F1, C], bf16, kind="ExternalInput")
    idx_dram = nc.dram_tensor("idxw", [2, 128, bpc // 16], i16,
                              kind="ExternalInput")
    w1t_dram = nc.dram_tensor("w1t", [2, 128, C], bf16, kind="ExternalInput")
    w2t_dram = nc.dram_tensor("w2t", [128, 2], bf16, kind="ExternalInput")
    b1_dram = nc.dram_tensor("b1t", [128, 2], f32, kind="ExternalInput")
    b3_dram = nc.dram_tensor("b3t", [1, 1], f32, kind="ExternalInput")
    id_dram = nc.dram_tensor("ident", [128, 128], bf16, kind="ExternalInput")
    out_dram = nc.dram_tensor("out", [bpc], f32, kind="ExternalOutput")
    out_view = out_dram.ap().rearrange("(c n) -> c n", n=chunk)

    with tile.TileContext(nc) as tc:
        with (
            tc.tile_pool(name="const", bufs=1) as const_pool,
            tc.tile_pool(name="gat", bufs=4) as gat_pool,
            tc.tile_pool(name="sum", bufs=2) as sum_pool,
            tc.tile_pool(name="h2", bufs=3) as h2_pool,
            tc.tile_pool(name="h3", bufs=3) as h3_pool,
            tc.tile_pool(name="stage", bufs=3) as stage_pool,
            tc.tile_pool(name="pt", bufs=3, space="PSUM") as pt_pool,
            tc.tile_pool(name="ph", bufs=3, space="PSUM") as ph_pool,
            tc.tile_pool(name="po", bufs=1, space="PSUM") as po_pool,
        ):
            # --- constants into SBUF; idx tiles first (split in groups
            # of 4 chunks) so the first gathers start ~immediately
            gsz = min(4, nchunk)  # chunks per idx tile
            gcw = gsz * (chunk // 16)
            ngidx = (bpc // 16) // gcw
            idxg = [[], []]
            for k in range(ngidx):
                for m in range(2):
                    it = const_pool.tile([128, gcw], i16, tag=f"idx{m}g{k}")
                    nc.sync.dma_start(
                        it[:], idx_dram.ap()[m, :, k * gcw:(k + 1) * gcw])
                    idxg[m].append(it)
            ident = const_pool.tile([128, 128], bf16)
            nc.sync.dma_start(ident[:], id_dram.ap())
            w1t = []
            for j in range(2):
                wt = const_pool.tile([128, C], bf16, tag=f"w1t{j}")
                nc.sync.dma_start(wt[:], w1t_dram.ap()[j])
                w1t.append(wt)
            w2t = const_pool.tile([128, 2], bf16)
            nc.sync.dma_start(w2t[:], w2t_dram.ap())
            b1s = const_pool.tile([128, 2], f32)
            nc.sync.dma_start(b1s[:], b1_dram.ap())
            b3s = const_pool.tile([1, 1], f32)
            nc.sync.dma_start(b3s[:], b3_dram.ap())

            def emit_fc2(pend):
                """fc2 + one batched final relu for a finished chunk
                (deferred so PE keeps streaming fc1 while Scalar drains
                h3; the [1,chunk] final costs one ACT instead of nblk)."""
                h3c, stage, ch = pend
                po = po_pool.tile([128, chunk], f32, tag="po")
                for blk in range(nblk):
                    psl = slice(blk * 512, (blk + 1) * 512)
                    for j in range(2):
                        nc.tensor.matmul(po[0:1, psl],
                                         w2t[:, j:j + 1], h3c[j][:, psl],
                                         start=(j == 0), stop=(j == 1))
                nc.scalar.activation(stage[0:1, :], po[0:1, :],
                                     Act.Relu, bias=b3s[0:1, 0:1])
                nc.sync.dma_start(out_view[ch:ch + 1, :], stage[:])

            cw = chunk // 16  # idx columns per chunk
            pend = None
            for ch in range(nchunk):
                isl = slice((ch % gsz) * cw, (ch % gsz + 1) * cw)
                # --- gather table rows: [128, ngrp, 256] (row layout)
                g = []
                for m, src in enumerate((q02_dram, q1_dram)):
                    dst = gat_pool.tile([128, ngrp, C], bf16, tag=f"g{m}")
                    nc.gpsimd.dma_gather(
                        dst[:], src.ap(), idxg[m][ch // gsz][:, isl],
                        chunk, chunk, C,
                        queue_num=(2 * ch + m) % NSWQ,
                    )
                    g.append(dst)

                s = sum_pool.tile([128, ngrp, C], bf16)
                nc.vector.tensor_tensor(s[:], g[0][:], g[1][:], Alu.add)
                stage = stage_pool.tile([1, chunk], f32)
                h3c = []
                for j in range(2):
                    h3t = h3_pool.tile([128, chunk], bf16, tag=f"h3{j}")
                    h3c.append(h3t)

                for blk in range(nblk):
                    bsl = slice(blk * 512, (blk + 1) * 512)
                    # --- transpose to [channel, batch]; with psumadd the
                    # two tables accumulate into the same PSUM region
                    h2 = []
                    for h in range(2):
                        ps = pt_pool.tile([128, 512], bf16, tag="pt")
                        for grp in range(4):
                            gi = 4 * blk + grp
                            csl = slice(h * 128, (h + 1) * 128)
                            osl = slice(grp * 128, (grp + 1) * 128)
                            nc.tensor.transpose(
                                ps[:, osl], s[:, gi, csl], ident[:])
                        # --- h2_h = relu(psum) on DVE (fast bf16 PSUM)
                        hs = h2_pool.tile([128, 512], bf16, tag=f"h2{h}")
                        nc.vector.tensor_scalar_max(hs[:], ps[:], 0.0)
                        h2.append(hs)
                    # --- fc1: ph_h = wfc1_h @ h2 + b1_h, k-chunked
                    for h in range(2):
                        ph = ph_pool.tile([128, 512], f32, tag="ph")
                        for j in range(2):
                            nc.tensor.matmul(
                                ph[:],
                                w1t[j][:, h * 128:(h + 1) * 128],
                                h2[j][:],
                                start=(j == 0), stop=(j == 1),
                            )
                        # --- h3_h = relu(ph_h + b1_h) on Scalar ACT
                        nc.scalar.activation(h3c[h][:, bsl], ph[:],
                                             Act.Relu, bias=b1s[:, h:h + 1])
                    if blk == 0 and pend is not None:
                        emit_fc2(pend)
                        pend = None
                pend = (h3c, stage, ch)
            emit_fc2(pend)

    nc.compile()
    _NC_CACHE[key] = nc
    return nc


def _fold_tables(inputs):
    """Q_m = relu(emb_m @ w1.T + b1) @ w2[:,:,m].T in float64, then the
    mode-0/2 pair fusion Q02[i*64+j] = Q0[i] + Q2[j] + b2."""
    w1_ = np.asarray(inputs["w1"]).astype(np.float64)
    b1_ = np.asarray(inputs["b1"]).astype(np.float64)
    w2 = np.asarray(inputs["w2"])
    qs = []
    for m, emb in enumerate((inputs["emb0"], inputs["emb1"], inputs["emb2"])):
        r = np.maximum(np.asarray(emb).astype(np.float64) @ w1_.T + b1_, 0.0)
        qs.append(r @ w2[:, :, m].astype(np.float64).T)
    q02 = (qs[0][:, None, :] + qs[2][None, :, :]
           + np.asarray(inputs["b2"]).astype(np.float64)).reshape(F02, C)
    return q02, qs[1]


def _make_common(inputs):
    bf = ml_dtypes.bfloat16
    q02, q1 = _fold_tables(inputs)
    return {
        "q02": np.ascontiguousarray(q02.astype(bf)),
        "q1": np.ascontiguousarray(q1.astype(bf)),
        "w1t": np.ascontiguousarray(
            np.asarray(inputs["wfc1"]).T.astype(bf).reshape(2, 128, C)),
        "w2t": np.ascontiguousarray(
            np.asarray(inputs["wfc2"]).reshape(C).astype(bf)
            .reshape(2, 128).T),
        "b1t": np.ascontiguousarray(
            np.asarray(inputs["bfc1"]).astype(np.float32).reshape(2, 128).T),
        "b3t": np.asarray(inputs["bfc2"]).astype(np.float32).reshape(1, 1),
        "ident": np.eye(128, dtype=bf),
    }


def _wrap_idx(idx, chunk):
    """Wrap a 1-D int array into dma_gather's [128, n/16] int16 layout,
    chunk by chunk: logical position k of chunk c lives at
    [k % 16, c*chunk/16 + k // 16], replicated across the 8 Q7 cores."""
    n = idx.shape[0]
    w = (idx.reshape(n // chunk, chunk // 16, 16)
         .transpose(0, 2, 1).reshape(n // chunk, 16, chunk // 16))
    wrapped = np.concatenate(list(w), axis=1).astype(np.int16)  # [16, n/16]
    return np.tile(wrapped, (8, 1))                             # [128, n/16]


def _make_idxw(shard, chunk=CHUNK):
    """shard: [n, 3] int indices -> ([2, 128, n/16] int16 wrapped layout,
    order) where row 0 is the fused mode-0/2 index and row 1 the mode-1
    index. The batch is sorted by the fused index so the big-table HBM
    reads ascend; `order` maps device position -> original row (undo with
    out[order] = device_out)."""
    i02 = np.asarray(shard[:, 0]).astype(np.int64) * FIELD_DIMS[2] \
        + np.asarray(shard[:, 2])
    i1 = np.asarray(shard[:, 1]).astype(np.int64)
    if SORT02:
        order = np.argsort(i02, kind="stable")
    else:
        order = np.arange(i02.shape[0])
    return np.stack([_wrap_idx(i02[order], chunk),
                     _wrap_idx(i1[order], chunk)]), order


def _run(inputs, trace=False, trace_kwargs=None):
    _install_ntff_hook()
    from concourse.bass_utils import run_bass_kernel_spmd

    nc = _build()
    common = _make_common(inputs)
    indices = np.asarray(inputs["indices"])
    in_maps, orders = [], []
    for c in range(N_CORES):
        shard = indices[c * BPC:(c + 1) * BPC]
        idxw, order = _make_idxw(shard)
        in_maps.append({**common, "idxw": idxw})
        orders.append(order)

    res = run_bass_kernel_spmd(nc, in_maps, core_ids=list(range(N_CORES)),
                               trace=trace, **(trace_kwargs or {}))
    out = np.empty(B, np.float32)
    for c in range(N_CORES):
        out[c * BPC + orders[c]] = res.results[c]["out"]
    return out, res


def kernel(**inputs):
    out, _ = _run(inputs, trace=False)
    return out
